# revision 17
# baseline (speedup 1.0000x reference)
import sys
import os

sys.path.insert(0, "/opt/trn_rl_repo")

import numpy as np

# Problem dims (hardcoded per spec)
B, T, E, H, V, K = 64, 512, 128, 256, 50000, 20
G4 = 4 * H                # 1024 gates per direction
NCORES = 8
RPC = 16                  # batch rows per core (and per pair)
CH = 8                    # rows per chain (2 chains per core)
XGB = 16                  # lstm steps per xg block (tokens per block = 16*XGB)


# ---------------------------------------------------------------------------
# Toolchain shim: this walrus build rejects instructions carrying more than
# ~2 sync waits ("Too many sync wait commands").  Split extra waits onto
# single-wait Drain carriers inserted just before the instruction.
# ---------------------------------------------------------------------------
def _install_shims():
    from concourse import bass_utils, bass2jax

    if getattr(bass_utils, "_wsplit_installed", False):
        return
    import orjson

    def _split_waits_bir(bir_json: bytes) -> bytes:
        d = orjson.loads(bir_json)
        ctr = 0
        changed = False
        for fn in d.get("functions", []):
            for blk in fn.get("blocks", []):
                out = []
                for ins in blk.get("instructions", []):
                    si = ins.get("sync_info")
                    if si:
                        ow = si.get("on_wait") or []
                        if len(ow) > 1:
                            changed = True
                            for w in ow[:-1]:
                                ctr += 1
                                nop = {
                                    "name": f"I-wsplit-{ctr}",
                                    "engine": ins["engine"],
                                    "opcode": "Drain",
                                    "ins": [],
                                    "outs": [],
                                    "sync_info": {"on_wait": [w], "on_update": []},
                                }
                                if "debug" in ins:
                                    nop["debug"] = ins["debug"]
                                out.append(nop)
                            si["on_wait"] = [ow[-1]]
                    out.append(ins)
                blk["instructions"] = out
        return orjson.dumps(d) if changed else bir_json

    orig = bass_utils.compile_bir_kernel

    def wrapped(bir_json, tmpdir, neff_name="file.neff"):
        return orig(_split_waits_bir(bir_json), tmpdir, neff_name)

    bass_utils.compile_bir_kernel = wrapped
    bass_utils._wsplit_installed = True
    bass2jax.compile_bir_kernel = wrapped


# ---------------------------------------------------------------------------
# Device kernel.  One SPMD program for all 8 cores.
# Core c: direction d=c//4 (0=fwd, 1=bwd), rows 16*(c%4) .. +16.
# Storage step s = processing order; fwd: time t=s, bwd: t=T-1-s.
# The bwd cores' emissions are un-reversed via the alpha/beta mask inputs
# before the pairwise AllReduce, so the summed emissions are time-ordered
# on every core.  Each core then runs the full 16-row Viterbi forward scan
# (redundant within a pair) and outputs the score history; the host
# backtraces.
# ---------------------------------------------------------------------------
def _build_nc(t_steps=T, em_debug=False):
    import concourse.bass as bass
    from concourse import mybir
    from concourse.tile import TileContext

    f32 = mybir.dt.float32
    bf16 = mybir.dt.float16  # fp16 everywhere: bf16 mantissa loses too many tags
    AF = mybir.ActivationFunctionType
    OP = mybir.AluOpType
    NT = RPC * t_steps            # tokens per core, s-major: tok = s*16 + r

    nc = bass.Bass()
    embT = nc.dram_tensor("embT", (E, NT), bf16, kind="ExternalInput")
    wihT = nc.dram_tensor("wihT", (E, G4), bf16, kind="ExternalInput")
    biasg = nc.dram_tensor("biasg", (128, 8), f32, kind="ExternalInput")
    whhT = nc.dram_tensor("whhT", (128, 2, G4), bf16, kind="ExternalInput")
    woutT = nc.dram_tensor("woutT", (128, 2, K), bf16, kind="ExternalInput")
    trp = nc.dram_tensor("trp", (RPC, K * K), f32, kind="ExternalInput")
    srow = nc.dram_tensor("srow", (RPC, K), f32, kind="ExternalInput")
    albe = nc.dram_tensor("albe", (CH, 2), mybir.dt.uint8, kind="ExternalInput")
    sch = nc.dram_tensor("sch", (RPC, t_steps, K), f32, kind="ExternalOutput")
    em_dbg = (
        nc.dram_tensor("em_dbg", (RPC, t_steps, K), f32, kind="ExternalOutput")
        if em_debug
        else None
    )

    with TileContext(nc) as tc:
        with (
            tc.tile_pool(name="const", bufs=1) as cp,
            tc.tile_pool(name="emb", bufs=1) as ep,
            tc.tile_pool(name="xg", bufs=2) as xp,
            tc.tile_pool(name="state", bufs=1) as stp,
            tc.tile_pool(name="work", bufs=2) as wp,
            tc.tile_pool(name="em", bufs=1) as emp,
            tc.tile_pool(name="ps", bufs=2, space="PSUM") as ps,
            tc.tile_pool(name="pse", bufs=1, space="PSUM") as pse,
            tc.tile_pool(name="dram", bufs=1, space="DRAM") as dp,
        ):
            # ---- constants / inputs resident in SBUF
            wih_sb = cp.tile([E, G4], bf16)
            nc.sync.dma_start(wih_sb[:], wihT[:])
            whh_sb = cp.tile([128, 2, G4], bf16)
            nc.sync.dma_start(whh_sb[:], whhT[:])
            wout_sb = cp.tile([128, 2, K], bf16)
            nc.sync.dma_start(wout_sb[:], woutT[:])
            biasg_sb = cp.tile([128, 8], f32)
            nc.sync.dma_start(biasg_sb[:], biasg[:])
            trp_sb = cp.tile([RPC, K * K], f32)
            nc.sync.dma_start(trp_sb[:], trp[:])
            srow_sb = cp.tile([RPC, K], f32)
            nc.sync.dma_start(srow_sb[:], srow[:])
            albe_sb = cp.tile([CH, 2], mybir.dt.uint8)
            nc.sync.dma_start(albe_sb[:], albe[:])

            # per-chain emission accumulators (rows on partitions 0-7)
            em_t = [emp.tile([CH, t_steps, K], f32, tag=f"em{x}", name=f"em{x}")
                     for x in (0, 1)]

            # LSTM state: c fp32, h bf16 ping-pong, laid out [p=h%128, cc, r]
            c_prev = [None, None]
            h_prev = [None, None]
            ep_ps = [None, None]   # emission psum, 8 steps per bank

            nblk = t_steps // XGB
            for blk in range(nblk):
                # ---- xg block: gates for steps blk*XGB .. +XGB (both chains)
                tok0 = blk * XGB * RPC
                ntok = XGB * RPC
                xg_blk = xp.tile([128, 8, ntok], f32, tag="xgblk", name=f"xgb{blk}", bufs=2)
                emb_blk = ep.tile([E, ntok], bf16, tag="embblk", name=f"embb{blk}", bufs=2)
                nc.sync.dma_start(emb_blk[:], embT[:, tok0:tok0 + ntok])
                for gc in range(8):
                    pxg = ps.tile([128, ntok], f32, tag="pxg", name=f"pxg{blk}_{gc}", bufs=2)
                    nc.tensor.matmul(
                        pxg[:],
                        wih_sb[:, gc * 128:(gc + 1) * 128],
                        emb_blk[:],
                        start=True, stop=True,
                    )
                    # psum -> sbuf bf16 with per-gate bias folded in
                    nc.scalar.activation(
                        xg_blk[:, gc, :], pxg[:], AF.Identity,
                        bias=biasg_sb[:, gc:gc + 1],
                    )

                for si in range(XGB):
                    s = blk * XGB + si
                    for ch in range(2):
                        rof = ch * CH
                        # gate preactivations for this chain: [p, gc, r]
                        xg_sl = xg_blk[:, :, si * RPC + rof: si * RPC + rof + CH]
                        if s > 0:
                            gps = ps.tile([128, 8, CH], f32, tag=f"gps{ch}", name=f"gps{ch}_{s}", bufs=2)
                            hp = h_prev[ch]
                            for gc in range(8):
                                for cc in range(2):
                                    nc.tensor.matmul(
                                        gps[:, gc, :],
                                        whh_sb[:, cc, gc * 128:(gc + 1) * 128],
                                        hp[:, cc, :],
                                        start=(cc == 0), stop=(cc == 1),
                                    )
                            gsum = wp.tile([128, 8, CH], f32, tag=f"gsum{ch}", name=f"gsum{ch}_{s}", bufs=2)
                            nc.vector.tensor_tensor(gsum[:], gps[:], xg_sl, OP.add)
                        else:
                            gsum = xg_sl
                        # activations: i,f sigmoid | g tanh | o sigmoid
                        ga = wp.tile([128, 8, CH], f32, tag=f"ga{ch}", name=f"ga{ch}_{s}", bufs=2)
                        if OLD_GATES:
                            nc.scalar.activation(ga[:, 0:4, :], gsum[:, 0:4, :], AF.Sigmoid)
                            nc.scalar.activation(ga[:, 4:6, :], gsum[:, 4:6, :], AF.Tanh)
                            nc.scalar.activation(ga[:, 6:8, :], gsum[:, 6:8, :], AF.Sigmoid)
                        else:
                            nc.scalar.activation(ga[:, 0:6, :], gsum[:, 0:6, :], AF.Sigmoid)
                            nc.scalar.activation(ga[:, 6:8, :], gsum[:, 6:8, :], AF.Tanh)
                        # c = f*c + i*g
                        t1 = wp.tile([128, 2, CH], f32, tag=f"t1{ch}", name=f"t1{ch}_{s}", bufs=2)
                        nc.vector.tensor_tensor(
                            t1[:], ga[:, 0:2, :],
                            ga[:, 4:6, :] if OLD_GATES else ga[:, 6:8, :], OP.mult
                        )
                        c_new = wp.tile([128, 2, CH], f32, tag=f"c{ch}", name=f"c{ch}_{s}", bufs=2)
                        if s == 0:
                            nc.vector.tensor_copy(c_new[:], t1[:])
                        else:
                            nc.vector.tensor_tensor(
                                c_new[:], ga[:, 2:4, :], c_prev[ch][:], OP.mult
                            )
                            nc.vector.tensor_tensor(
                                c_new[:], c_new[:], t1[:], OP.add
                            )
                        c_prev[ch] = c_new
                        # h = o * tanh(c)
                        th = wp.tile([128, 2, CH], f32, tag=f"th{ch}", name=f"th{ch}_{s}", bufs=2)
                        nc.scalar.activation(th[:], c_new[:], AF.Tanh)
                        h_new = wp.tile([128, 2, CH], bf16, tag=f"h{ch}", name=f"h{ch}_{s}", bufs=2)
                        nc.vector.tensor_tensor(
                            h_new[:],
                            ga[:, 6:8, :] if OLD_GATES else ga[:, 4:6, :],
                            th[:], OP.mult,
                        )
                        h_prev[ch] = h_new
                        # emissions: ep[r, s%8, k] += h_cc @ woutT_cc
                        if s % 8 == 0:
                            ep_ps[ch] = pse.tile([CH, 8, K], f32, tag=f"ep{ch}", name=f"ep{ch}_{s}", bufs=1)
                        for cc in range(2):
                            nc.tensor.matmul(
                                ep_ps[ch][:, s % 8, :],
                                h_new[:, cc, :],
                                wout_sb[:, cc, :],
                                start=(cc == 0), stop=(cc == 1),
                            )
                        if s % 8 == 7:
                            nc.scalar.copy(
                                em_t[ch][:, s - 7:s + 1, :], ep_ps[ch][:]
                            )

            # ---- un-reverse (select by direction mask) + pairwise AllReduce
            bounce = dp.tile([RPC, t_steps, K], f32, name="bounce")
            red = dp.tile([RPC, t_steps, K], f32, name="red")
            for ch in range(2):
                rof = ch * CH
                ct = wp.tile([CH, t_steps, K], f32, tag="ct", name=f"ct{ch}", bufs=1)
                mask_b = albe_sb[:, 0:1].unsqueeze(1).broadcast_to(
                    (CH, t_steps, K)
                )
                em_rev = em_t[ch][:, ::-1, :]
                nc.vector.select(ct[:], mask_b, em_t[ch][:], em_rev)
                nc.sync.dma_start(bounce[rof:rof + CH, :, :], ct[:])
            nc.gpsimd.collective_compute(
                "AllReduce",
                mybir.AluOpType.add,
                replica_groups=[[0, 4], [1, 5], [2, 6], [3, 7]],
                ins=[bounce[:].opt()],
                outs=[red[:].opt()],
            )
            em_full = emp.tile([RPC, t_steps, K], f32, name="em_full")
            nc.sync.dma_start(em_full[:], red[:])
            if em_debug:
                nc.sync.dma_start(em_dbg[:], red[:])

            # ---- Viterbi forward max-scan, batch rows on partitions.
            # Scores overwrite em_full in place: step t reads em[:, t, :] and
            # writes score_t to the same slice (DVE reads ahead of writes).
            sc = em_full
            nc.vector.tensor_tensor(
                sc[:, 0, :], em_full[:, 0, :], srow_sb[:], OP.add
            )
            trv = trp_sb[:].rearrange("p (j i) -> p j i", j=K)
            for t in range(1, t_steps):
                cand = wp.tile([RPC, K, K], f32, tag="cand", name=f"cand{t}", bufs=2)
                prev = sc[:, t - 1, :].unsqueeze(1).broadcast_to((RPC, K, K))
                nc.vector.tensor_tensor(cand[:], prev, trv, OP.add)
                m = wp.tile([RPC, K], f32, tag="vm", name=f"vm{t}", bufs=2)
                nc.vector.tensor_reduce(
                    m[:], cand[:], mybir.AxisListType.X, OP.max
                )
                nc.vector.tensor_tensor(
                    sc[:, t, :], m[:], em_full[:, t, :], OP.add
                )
            nc.sync.dma_start(sch[:], sc[:])
    return nc


# ---------------------------------------------------------------------------
# Host-side input prep / launch / backtrace
# ---------------------------------------------------------------------------
OLD_GATES = bool(os.environ.get("KM_OLD_GATES"))
GPERM = (np.arange(G4) if OLD_GATES else
         np.concatenate([np.arange(0, 2 * H), np.arange(3 * H, 4 * H),
                         np.arange(2 * H, 3 * H)]))  # (i, f, o, g) gate order


def _prep_inputs(emb, Wih_f, Whh_f, b_f, Wih_b, Whh_b, b_b, Wout, bout,
                 start_trans, transitions, t_steps=T):
    def chunk2(a):  # (256, n) -> (128, 2, n)
        return np.ascontiguousarray(
            a.reshape(2, 128, -1).transpose(1, 0, 2)
        )

    trp_m = np.ascontiguousarray(
        (transitions + bout[None, :]).T
    ).reshape(1, K * K).repeat(RPC, axis=0).astype(np.float32)
    srow_m = (start_trans + bout).reshape(1, K).repeat(RPC, axis=0).astype(np.float32)

    in_maps = []
    for c in range(NCORES):
        d = c // 4
        r0 = (c % 4) * RPC
        Wih, Whh, bg = (Wih_f, Whh_f, b_f) if d == 0 else (Wih_b, Whh_b, b_b)
        Wih, Whh, bg = Wih[GPERM], Whh[GPERM], bg[GPERM]
        e = emb[r0:r0 + RPC, :t_steps]            # (16, t, E), time order
        if d == 1:
            e = e[:, ::-1]                         # storage s = reversed time
        # embT[e_dim, s*16 + r]
        embT = np.ascontiguousarray(
            e.transpose(2, 1, 0).reshape(E, t_steps * RPC)
        ).astype(np.float32)
        albe = np.zeros((CH, 2), np.uint8)
        albe[:, 0] = 1 - d
        in_maps.append({
            "embT": embT,
            "wihT": np.ascontiguousarray(Wih.T).astype(np.float32),
            "biasg": np.ascontiguousarray(bg.reshape(8, 128).T).astype(np.float32),
            "whhT": chunk2(np.ascontiguousarray(Whh.T)).astype(np.float32),
            "woutT": chunk2(
                np.ascontiguousarray(Wout[:, d * H:(d + 1) * H].T)
            ).astype(np.float32),
            "trp": trp_m,
            "srow": srow_m,
            "albe": albe,
        })
    return in_maps


def _to_bf16(in_maps):
    for m in in_maps:
        for k in ("embT", "wihT", "whhT", "woutT"):
            m[k] = m[k].astype(np.float16)
    return in_maps


def _backtrace(scores, transitions, end_trans, t_steps=T):
    # scores: (B, T, K) fwd viterbi scores; replicate pytorch-crf backtrace
    b = scores.shape[0]
    tags = np.empty((b, t_steps), np.int32)
    last = np.argmax(scores[:, t_steps - 1, :] + end_trans[None, :], axis=-1)
    tags[:, t_steps - 1] = last
    tr = transitions.astype(np.float64)
    sc = scores.astype(np.float64)
    cur = last
    for t in range(t_steps - 2, -1, -1):
        cand = sc[:, t, :] + tr[:, cur].T       # (B, K)
        cur = np.argmax(cand, axis=-1)
        tags[:, t] = cur
    return tags


def kernel(x, mask, embedding, Wih_f, Whh_f, b_f, Wih_b, Whh_b, b_b,
           Wout, bout, start_trans, end_trans, transitions):
    _install_shims()
    from concourse.bass_utils import run_bass_kernel_spmd

    x = np.asarray(x)
    embedding = np.asarray(embedding, np.float32)
    emb = embedding[np.asarray(x, np.int64)]      # (B, T, E)

    in_maps = _prep_inputs(
        emb,
        np.asarray(Wih_f, np.float32), np.asarray(Whh_f, np.float32),
        np.asarray(b_f, np.float32),
        np.asarray(Wih_b, np.float32), np.asarray(Whh_b, np.float32),
        np.asarray(b_b, np.float32),
        np.asarray(Wout, np.float32), np.asarray(bout, np.float32),
        np.asarray(start_trans, np.float32), np.asarray(transitions, np.float32),
    )
    in_maps = _to_bf16(in_maps)

    nc = _build_nc()
    res = run_bass_kernel_spmd(nc, in_maps, core_ids=list(range(NCORES)))

    scores = np.empty((B, T, K), np.float32)
    for c in range(4):
        r0 = c * RPC
        scores[r0:r0 + RPC] = res.results[c]["sch"]
    if os.environ.get("KM_SAVE_SCORES"):
        np.save(os.environ["KM_SAVE_SCORES"], scores)
    tags = _backtrace(
        scores, np.asarray(transitions, np.float32),
        np.asarray(end_trans, np.float32),
    )
    return tags.astype(np.int32)
